# revision 22
# baseline (speedup 1.0000x reference)
"""Additive (Bahdanau-style) attention kernel for Trainium2, 8 NeuronCores.

reference computation (per batch b of 32, T=1024 timesteps, H=1024):
    mlp_hidden = selu([enc[b,t]; hid[b]] @ W1 + b1)     # (T, H)
    scores     = mlp_hidden @ W2 + b2                   # (T, 1)
    weights    = softmax(scores over t)
    out[b]     = sum_t weights[t] * enc[b,t]            # (H,)

Distribution: data-parallel over batch, 4 batches per core, no collectives.

Per-core algorithm (shard shapes):
  - enc @ W1 splits: enc @ W1[:H] + hid @ W1[H:]; the second term ("hid_part")
    is per-batch constant, computed once as a tiny matmul.
  - b2 and selu's additive constant are dropped (softmax shift invariance);
    selu's lambda is folded into W2 on the host.
  - all TensorE operands are bf16 (cast for free by the gpsimd DMAs on load,
    or written as bf16 by ScalarE/VectorE): weight loads use the fast path
    and matmuls run at 1 cycle/row.  The f32 encoder tensor is still read
    from HBM in full, once.
  - scores path: PE-transpose E tiles -> psum[j,t] accumulating
    W1a-chunk^T @ E^T-chunk -> selu via 2 ScalarE + 2 VectorE passes ->
    per-j-chunk dot with W2 on TensorE (psum f32 accumulation throughout).
  - softmax over the 1024 scores on one partition row.
  - context = w @ E with E SBUF-resident (single HBM read of the encoder).
"""

import numpy as np

import concourse.bass as bass
import concourse.tile as tile
from concourse import bacc, mybir
from concourse.bass_utils import run_bass_kernel_spmd
from concourse.masks import make_identity

F32 = mybir.dt.float32
BF16 = mybir.dt.bfloat16
AX = mybir.AxisListType
ALU = mybir.AluOpType
ACTF = mybir.ActivationFunctionType

N_CORES = 8
B = 32
T = 1024
H = 1024
BL = B // N_CORES          # batches per core = 4
KC = H // 128              # contraction chunks = 8
JC = H // 128              # hidden-unit chunks = 8
TGS = 512                  # t-group size (one psum bank of f32)
NTG = T // TGS             # t-groups per batch = 2
TT = TGS // 128            # 128-row t-subtiles per t-group = 4

SELU_LAMBDA = 1.0507009873554805
SELU_ALPHA = 1.6732632423543772


def build_kernel():
    nc = bacc.Bacc("TRN2", target_bir_lowering=False, debug=False,
                   num_devices=N_CORES)

    enc = nc.dram_tensor("enc", [BL, KC, 128, H], F32, kind="ExternalInput").ap()
    hidT = nc.dram_tensor("hidT", [KC, 128, BL], F32, kind="ExternalInput").ap()
    w1a = nc.dram_tensor("w1a", [KC, 128, H], F32, kind="ExternalInput").ap()
    w1b = nc.dram_tensor("w1b", [KC, 128, H], F32, kind="ExternalInput").ap()
    b1r = nc.dram_tensor("b1r", [128, JC], F32, kind="ExternalInput").ap()
    w2l = nc.dram_tensor("w2l", [128, JC], F32, kind="ExternalInput").ap()
    out = nc.dram_tensor("out", [BL, H], F32, kind="ExternalOutput").ap()

    with tile.TileContext(nc) as tc:
        with (
            tc.tile_pool(name="consts", bufs=1) as consts,
            tc.tile_pool(name="encp", bufs=3) as encp,
            tc.tile_pool(name="etp", bufs=2) as etp,
            tc.tile_pool(name="selu", bufs=3) as selup,
            tc.tile_pool(name="score", bufs=2) as scorep,
            tc.tile_pool(name="outp", bufs=2) as outp,
            tc.tile_pool(name="psum", bufs=2, space="PSUM") as psum,
            tc.tile_pool(name="dram", bufs=2, space="DRAM") as dram,
        ):
            # identity + PE warmup first: the warmup transposes depend only
            # on the identity tile, so the TensorE activity monitor sees work
            # from ~1us and the clock ungates before real matmuls arrive.
            identity = consts.tile([128, 128], BF16)
            make_identity(nc, identity)
            one1 = consts.tile([1, 1], F32)
            nc.vector.memset(one1, 1.0)
            warm_ps = psum.tile([128, 128], BF16, tag="mlp", bufs=4)
            for _ in range(32):
                nc.tensor.transpose(warm_ps, identity, identity)

            # --- replicated weights (casting gpsimd DMAs, W1b first) ----
            b1r_sb = consts.tile([128, JC], F32)
            nc.sync.dma_start(out=b1r_sb, in_=b1r)
            w1a_sb = consts.tile([128, KC, H], BF16)
            w1b_sb = consts.tile([128, KC, H], BF16)
            for k in range(KC):
                nc.gpsimd.dma_start(out=w1b_sb[:, k, :], in_=w1b[k])
            hidT_sb = consts.tile([128, KC, BL], BF16)
            nc.gpsimd.dma_start(out=hidT_sb, in_=hidT.rearrange("k p b -> p k b"))
            w2l_sb = consts.tile([128, JC], BF16)
            nc.gpsimd.dma_start(out=w2l_sb, in_=w2l)
            for k in range(KC):
                nc.gpsimd.dma_start(out=w1a_sb[:, k, :], in_=w1a[k])

            hid_ps = psum.tile([128, JC, BL], F32, tag="ctx", bufs=1)
            for j in range(JC):
                for k in range(KC):
                    nc.tensor.matmul(
                        hid_ps[:, j, :],
                        lhsT=w1b_sb[:, k, j * 128:(j + 1) * 128],
                        rhs=hidT_sb[:, k, :],
                        start=(k == 0),
                        stop=(k == KC - 1),
                    )
            hb = consts.tile([128, JC, BL], F32)
            for j in range(JC):
                nc.vector.tensor_scalar(
                    out=hb[:, j, :], in0=hid_ps[:, j, :],
                    scalar1=b1r_sb[:, j:j + 1], scalar2=None, op0=ALU.add,
                )

            for b in range(BL):
                # ---- load E for this batch: [t_in_tile, t_tile, h] ----
                # batch 0 rides the HWDGE ring as f32 (VectorE casts) so both
                # DMA paths stream in parallel during startup; later batches
                # use the casting gpsimd DMAs.
                e_sb = encp.tile([128, KC, H], BF16, tag="e")
                if b == 0:
                    e_f = encp.tile([128, KC, H], F32, tag="ef", bufs=1)
                    for tt in range(KC):
                        nc.sync.dma_start(out=e_f[:, tt, :], in_=enc[b, tt])
                        nc.vector.tensor_copy(out=e_sb[:, tt, :],
                                              in_=e_f[:, tt, :])
                else:
                    for tt in range(KC):
                        nc.gpsimd.dma_start(
                            out=e_sb[:, tt, :], in_=enc[b, tt])

                scores_row = scorep.tile([1, T], F32, tag="scores")

                for tg in range(NTG):
                    # ---- E^T tiles for this t-group: [h_in_chunk, k, t] ----
                    eT_sb = etp.tile([128, KC, TGS], BF16, tag="eT", bufs=4)
                    for k in range(KC):
                        tp = psum.tile([128, TGS], BF16, tag="trans")
                        for tt in range(TT):
                            t_idx = tg * TT + tt
                            nc.tensor.transpose(
                                tp[:, tt * 128:(tt + 1) * 128],
                                e_sb[:, t_idx, k * 128:(k + 1) * 128],
                                identity,
                            )
                        nc.vector.tensor_copy(out=eT_sb[:, k, :], in_=tp)

                    # ---- mlp + selu + score dot, per j-chunk ----
                    sc_ps = psum.tile([1, TGS], F32, tag="sc", bufs=1)
                    for j in range(JC):
                        mp = psum.tile([128, TGS], F32, tag="mlp", bufs=4)
                        for k in range(KC):
                            nc.tensor.matmul(
                                mp,
                                lhsT=w1a_sb[:, k, j * 128:(j + 1) * 128],
                                rhs=eT_sb[:, k, :],
                                start=(k == 0),
                                stop=(k == KC - 1),
                            )
                        bias_ap = hb[:, j, b:b + 1]
                        e2 = selup.tile([128, TGS], BF16, tag="e2")
                        nc.scalar.activation(out=e2, in_=mp, func=ACTF.Exp,
                                             bias=bias_ap, scale=1.0)
                        r2 = selup.tile([128, TGS], BF16, tag="r2")
                        nc.scalar.activation(out=r2, in_=mp, func=ACTF.Relu,
                                             bias=bias_ap, scale=1.0)
                        # in place: e2 <- alpha * min(e2, 1)
                        nc.vector.tensor_scalar(
                            out=e2, in0=e2, scalar1=1.0, scalar2=SELU_ALPHA,
                            op0=ALU.min, op1=ALU.mult,
                        )
                        # s2 <- r2 + e2, cast to bf16 on the DVE write
                        s2 = selup.tile([128, TGS], BF16, tag="s2")
                        nc.vector.tensor_add(out=s2, in0=r2, in1=e2)
                        nc.tensor.matmul(
                            sc_ps,
                            lhsT=w2l_sb[:, j:j + 1],
                            rhs=s2,
                            start=(j == 0),
                            stop=(j == JC - 1),
                        )
                    nc.scalar.copy(out=scores_row[:, tg * TGS:(tg + 1) * TGS],
                                   in_=sc_ps)

                # ---- softmax over t on one partition row ----
                rmax = scorep.tile([1, 1], F32, tag="rmax")
                nc.vector.tensor_reduce(out=rmax, in_=scores_row, axis=AX.X,
                                        op=ALU.max)
                nmax = scorep.tile([1, 1], F32, tag="nmax")
                nc.vector.tensor_scalar_mul(nmax, rmax, -1.0)
                expw = scorep.tile([1, T], F32, tag="expw")
                rsum = scorep.tile([1, 1], F32, tag="rsum")
                nc.scalar.activation(out=expw, in_=scores_row, func=ACTF.Exp,
                                     bias=nmax, scale=1.0, accum_out=rsum)
                rinv = scorep.tile([1, 1], F32, tag="rinv")
                nc.vector.reciprocal(rinv, rsum)

                # ---- unnormalized weights -> column chunks (PE transpose);
                # normalization by 1/Z is folded into the output copy.
                w_ps = psum.tile([128, KC], F32, tag="ctx", bufs=1)
                for c in range(KC):
                    nc.tensor.transpose(
                        w_ps[:, c:c + 1],
                        expw[0:1, c * 128:(c + 1) * 128],
                        one1,
                    )
                w_col = scorep.tile([128, KC], BF16, tag="wcol")
                nc.vector.tensor_copy(out=w_col, in_=w_ps)

                # ---- context[h] = (sum_t exp[t] * E[t, h]) / Z ----
                ob = outp.tile([1, H], F32, tag="ob")
                for half in range(2):
                    cp = psum.tile([1, TGS], F32, tag="ctx", bufs=1)
                    for tch in range(KC):
                        nc.tensor.matmul(
                            cp,
                            lhsT=w_col[:, tch:tch + 1],
                            rhs=e_sb[:, tch, half * TGS:(half + 1) * TGS],
                            start=(tch == 0),
                            stop=(tch == KC - 1),
                        )
                    nc.scalar.activation(
                        out=ob[:, half * TGS:(half + 1) * TGS], in_=cp,
                        func=ACTF.Copy, scale=rinv)
                nc.gpsimd.dma_start(out=out[b:b + 1, :], in_=ob)

    nc.compile()
    return nc


_NC_CACHE = None


def _get_nc():
    global _NC_CACHE
    if _NC_CACHE is None:
        _NC_CACHE = build_kernel()
    return _NC_CACHE


def make_in_maps(encoder_outputs, hidden_state, W1, b1, W2):
    enc = np.ascontiguousarray(np.asarray(encoder_outputs, np.float32))
    hid = np.ascontiguousarray(np.asarray(hidden_state, np.float32))
    W1 = np.asarray(W1, np.float32)
    b1 = np.asarray(b1, np.float32)
    W2 = np.asarray(W2, np.float32)

    w1a = np.ascontiguousarray(W1[:H]).reshape(KC, 128, H)
    w1b = np.ascontiguousarray(W1[H:]).reshape(KC, 128, H)
    b1r = np.ascontiguousarray(b1.reshape(KC, 128).T)               # (128, KC)
    w2l = np.ascontiguousarray((W2[:, 0] * SELU_LAMBDA).reshape(JC, 128).T)

    in_maps = []
    for c in range(N_CORES):
        sl = slice(BL * c, BL * (c + 1))
        in_maps.append({
            "enc": np.ascontiguousarray(enc[sl]).reshape(BL, KC, 128, H),
            "hidT": np.ascontiguousarray(hid[0, sl].T).reshape(KC, 128, BL),
            "w1a": w1a,
            "w1b": w1b,
            "b1r": b1r,
            "w2l": w2l,
        })
    return in_maps


def kernel(encoder_outputs, hidden_state, W1, b1, W2, b2):
    # b2 shifts every score equally; softmax is shift-invariant, so it is
    # deliberately unused.
    in_maps = make_in_maps(encoder_outputs, hidden_state, W1, b1, W2)
    nc = _get_nc()
    res = run_bass_kernel_spmd(nc, in_maps, core_ids=list(range(N_CORES)))
    out = np.empty((1, B, H), np.float32)
    for c in range(N_CORES):
        out[0, BL * c:BL * (c + 1)] = res.results[c]["out"]
    return out
